# revision 32
# baseline (speedup 1.0000x reference)
"""Fused BigVGAN Activation1d (upsample2x -> SnakeBeta -> downsample2x) on
8 Trainium2 NeuronCores.

Strategy (validated in golden.py):
  - Channel-sharded SPMD: 512 channels / 8 cores = 64 channels per core,
    16 batch rows each, T=8192. No cross-core communication.
  - All FIRs are banded-matrix matmuls on the TensorEngine in a
    time-in-partitions layout; the host bakes overlapped clamped x
    windows (stride NB=115, length 128, row 127 = constant 1.0) so the
    device never transposes and never special-cases edges.
  - Math:  act = up + rb*sin^2(a*up),  a=exp(alpha), rb=1/(exp(beta)+1e-9)
        out = D(act) = H(x) + D((rb/2)*(-cos(2a*up))) + (rb/2)*sum(d)
    The up-matmul emits w' = (a/pi)*u + 96.25 (per-channel stationaries,
    offset via the const row).  W' lives in [64,128) so frac(w') is the
    low 17 mantissa bits: one DVE bitwise_and extracts it; ScalarE
    computes c = Sin(2pi*frac - pi) = -cos(2a*u); GPSIMD applies the
    per-channel rb/2; H/D matmuls accumulate in PSUM; the final
    PSUM->SBUF copy adds (rb/2)*sum(d).
"""
import numpy as np
from contextlib import ExitStack

import concourse.bacc as bacc
import concourse.tile as tile
from concourse import mybir
from concourse.bass_utils import run_bass_kernel_spmd

# ---- problem geometry (hardcoded per spec) --------------------------------
B, C, T = 16, 512, 8192
NCORES = 8
CPC = C // NCORES          # 64 channels per core
K = 12
NB = 115                   # out samples per block
NBLK = (T + NB - 1) // NB  # 72
XW, UW = 128, 121
XOFF, UOFF = -6, -3
COLS = NBLK * B            # 1152 free columns per channel (block-major, batch)
OFFSET = 192.25            # binade [128,256): frac = the low int16 of the f32
FRACBITS = 16

F16 = mybir.dt.float16
F32 = mybir.dt.float32
I32 = mybir.dt.int32
I16 = mybir.dt.int16

# column chunking for PSUM tiles
CHUNKS = [(0, 512), (512, 512), (1024, 128)]
# H/D stationary variant splits over columns: (start, end, variant)
SPLITS = [(0, B, 0), (B, (NBLK - 1) * B, 1), ((NBLK - 1) * B, COLS, 2)]
# shared w_pack column offsets: H[3] | De[3] | Do[3], each padded to 128
# columns so LDWEIGHTS runs with a full 128-wide stationary (FWL-eligible)
NBP = 128
W_H = [NBP * k for k in range(3)]
W_DE = [3 * NBP + NBP * k for k in range(3)]
W_DO = [6 * NBP + NBP * k for k in range(3)]
WCOLS = 9 * NBP
# base (unscaled) up stationaries, shared: ue | uo, padded to 128 cols
UWP = 128
UBCOLS = 2 * UWP
# tbl columns: a/pi | A=rb/2 | BS | -pi
TBLCOLS = 3 * CPC + 1


# ---------------------------------------------------------------------------
# host-side constant builders (validated in golden.py)
# ---------------------------------------------------------------------------

def _phase_filters(up_filter):
    f = up_filter.astype(np.float64)
    fe = np.array([2.0 * f[11 - 2 * j] for j in range(6)])
    fo = np.array([2.0 * f[10 - 2 * j] for j in range(6)])
    return fe, fo


def _build_up_stationaries(fe, fo):
    W_ue = np.zeros((XW, UW))
    W_uo = np.zeros((XW, UW))
    for q in range(UW):
        for k in range(6):
            W_ue[q + k, q] += fe[k]
            W_uo[q + k + 1, q] += fo[k]
    return W_ue, W_uo


def _build_down_stationaries(d_full, block):
    D_We = np.zeros((XW, NB))
    D_Wo = np.zeros((XW, NB))
    u0 = NB * block + UOFF
    for o in range(NB):
        n = NB * block + o
        if n >= T:
            continue
        for t in range(K):
            c = min(max(2 * n + t - 5, 0), 2 * T - 1)
            q = c // 2 - u0
            if c % 2 == 0:
                D_We[q, o] += d_full[t]
            else:
                D_Wo[q, o] += d_full[t]
    return D_We, D_Wo


def _build_h_stationary(fe, fo, d_full, block):
    H = np.zeros((XW, NB))
    x0 = NB * block + XOFF
    for o in range(NB):
        n = NB * block + o
        if n >= T:
            continue
        for t in range(K):
            c = min(max(2 * n + t - 5, 0), 2 * T - 1)
            u = c // 2
            if c % 2 == 0:
                base, taps = u - 3, fe
            else:
                base, taps = u - 2, fo
            for k in range(6):
                xg = min(max(base + k, -5), T + 4)
                H[xg - x0, o] += d_full[t] * taps[k]
    return H


def _host_constants(up_filter, down_filter):
    fe, fo = _phase_filters(up_filter)
    d_full = down_filter.astype(np.float64)
    w = np.zeros((XW, WCOLS))
    for k, blk in enumerate((0, 1, NBLK - 1)):
        w[:, W_H[k]:W_H[k] + NB] = _build_h_stationary(fe, fo, d_full, blk)
        De, Do = _build_down_stationaries(d_full, blk)
        w[:, W_DE[k]:W_DE[k] + NB] = De
        w[:, W_DO[k]:W_DO[k] + NB] = Do
    W_ue, W_uo = _build_up_stationaries(fe, fo)
    ub = np.zeros((XW, UBCOLS))
    ub[:, :UW] = W_ue
    ub[:, UWP:UWP + UW] = W_uo
    ub[127, :] = OFFSET    # scaled by tbl row 127 == 1.0 on the uch build
    S = float(np.sum(d_full))
    return (np.ascontiguousarray(w.astype(np.float16)),
            np.ascontiguousarray(ub.astype(np.float16)), S)


GRP = 4                     # channels per DMA group
NGRP = CPC // GRP           # 16 groups per core


def _prep_x_core(x_core):
    """x_core: [B, CPC, T] f32 -> x_dev [NGRP, XW, GRP, COLS] fp16 windows,
    grouped so one DMA loads GRP channels."""
    idx = np.arange(NBLK)[:, None] * NB + XOFF + np.arange(XW)[None, :]
    idx = np.clip(idx, 0, T - 1)
    wins = x_core[:, :, idx]                      # [B, CPC, NBLK, XW]
    x_dev = wins.transpose(1, 3, 2, 0).reshape(CPC, XW, COLS).astype(np.float16)
    x_dev[:, 127, :] = np.float16(1.0)
    x_dev = x_dev.reshape(NGRP, GRP, XW, COLS).transpose(0, 2, 1, 3)
    return np.ascontiguousarray(x_dev)


# ---------------------------------------------------------------------------
# device kernel
# ---------------------------------------------------------------------------

def build_nc(repeat=0, ablate=()):
    """ablate: subset of {"in_dma","out_dma","act","up_mm","down_mm","dve"}
    — drops that component (garbage numerics) for HW bottleneck attribution."""
    ablate = frozenset(ablate)
    nc = bacc.Bacc("TRN2", target_bir_lowering=False, debug=False,
                   num_devices=NCORES)
    x_d = nc.declare_dram_parameter("x_dev", [NGRP, XW, GRP * COLS], F16,
                                    isOutput=False)
    w_d = nc.declare_dram_parameter("w_pack", [XW, WCOLS], F16, isOutput=False)
    u_d = nc.declare_dram_parameter("u_base", [XW, UBCOLS], F16, isOutput=False)
    t_d = nc.declare_dram_parameter("tbl", [XW, TBLCOLS], F32, isOutput=False)
    o_d = nc.declare_dram_parameter("out_dev", [NGRP, NB, GRP * COLS], F16,
                                    isOutput=True)

    SIN_SCALE = float(2.0 * np.pi / (1 << FRACBITS))

    with tile.TileContext(nc) as tc, ExitStack() as ctx:
        wp = ctx.enter_context(tc.tile_pool(name="wp", bufs=1))
        xp = ctx.enter_context(tc.tile_pool(name="xp", bufs=4))
        cp = ctx.enter_context(tc.tile_pool(name="cp", bufs=4))
        gp = ctx.enter_context(tc.tile_pool(name="gp", bufs=4))
        op = ctx.enter_context(tc.tile_pool(name="op", bufs=2))
        pu = ctx.enter_context(tc.tile_pool(name="pu", bufs=3, space="PSUM"))
        po = ctx.enter_context(tc.tile_pool(name="po", bufs=2, space="PSUM"))

        wt = wp.tile([XW, WCOLS], F16)
        nc.sync.dma_start(wt[:], w_d[:])
        ub = wp.tile([XW, UBCOLS], F16)
        nc.sync.dma_start(ub[:], u_d[:])
        tb = wp.tile([XW, TBLCOLS], F32)
        nc.sync.dma_start(tb[:], t_d[:])

        # per-channel scaled up stationaries: rows 0..126 = (a/pi)*base,
        # row 127 = OFFSET (tbl row 127 of the a/pi block is 1.0)
        uch = wp.tile([XW, CPC, UBCOLS], F16)
        uch_d = wp.tile([XW, CPC, 2 * NBP], F16)

        def build_uch(ch):
            # just-in-time per-channel stationary scaling, all on the
            # otherwise-idle GPSIMD so ACT/DVE queues stay clean; issued
            # several channels ahead of first use.
            n = 4 if "build" in ablate else None
            nc.gpsimd.tensor_scalar_mul(uch[:, ch, 0:n or UBCOLS],
                                        ub[:, 0:n or UBCOLS],
                                        tb[:, ch:ch + 1])
            nc.vector.tensor_scalar_mul(
                uch_d[:, ch, 0:n or NBP],
                wt[:, W_DE[1]:W_DE[1] + (n or NBP)],
                tb[:, CPC + ch:CPC + ch + 1])
            nc.vector.tensor_scalar_mul(
                uch_d[:, ch, NBP:NBP + (n or NBP)],
                wt[:, W_DO[1]:W_DO[1] + (n or NBP)],
                tb[:, CPC + ch:CPC + ch + 1])

        # Chunk-granular software pipeline: PE alternates up-chunk(ch,k)
        # with down-chunk(ch-1,k), so ScalarE sin of chunk k overlaps the
        # PE's down-stage matmuls and no engine waits a full channel.
        stash = {}

        def up_chunk(ch, ci):
            c0, w = CHUNKS[ci]
            g, chi = divmod(ch, GRP)
            if chi == 0 and ci == 0:
                xg = xp.tile([XW, GRP * COLS], F16, tag="xt")
                if "in_dma" not in ablate:
                    nc.sync.dma_start(xg[:], x_d[g])
                else:
                    nc.gpsimd.memset(xg[:, 0:4], 1.0)
                stash[("xg", g)] = xg
            if ci == 0:
                ct = cp.tile([UW, 2 * COLS], F16, tag="ct")
                stash[ch] = (stash[("xg", g)], ct, None)
            xg, ct, _ = stash[ch]
            xb = chi * COLS
            ps = pu.tile([XW, 1024], F32, tag="pu")
            if "up_mm" not in ablate:
                nc.tensor.matmul(ps[:, 0:w], uch[:, ch, 0:UWP],
                                 xg[:, xb + c0:xb + c0 + w],
                                 start=True, stop=True)
                nc.tensor.matmul(ps[:, w:2 * w], uch[:, ch, UWP:UBCOLS],
                                 xg[:, xb + c0:xb + c0 + w],
                                 start=True, stop=True)
            else:
                nc.tensor.matmul(ps[:, 0:16], uch[:, ch, 0:UWP],
                                 xg[:, xb + c0:xb + c0 + 16],
                                 start=True, stop=True)
            lo16 = ps[0:UW, 0:2 * w].bitcast(I16)[:, 0::2]
            if "act" not in ablate:
                nc.scalar.activation(ct[:, 2 * c0:2 * c0 + 2 * w], lo16,
                                     mybir.ActivationFunctionType.Sin,
                                     bias=0.0, scale=SIN_SCALE)
            else:
                nc.scalar.activation(ct[:, 2 * c0:2 * c0 + 8], lo16[:, 0:8],
                                     mybir.ActivationFunctionType.Sin,
                                     bias=0.0, scale=SIN_SCALE)
            if ci == len(CHUNKS) - 1:
                # edge-block g = -(rb/2)*c: [ge_j0|go_j0|ge_j71|go_j71]
                ge = gp.tile([UW, 4 * B], F16, tag="ge")
                acol = tb[0:UW, CPC + ch:CPC + ch + 1]
                if "ge" in ablate:
                    nc.gpsimd.tensor_scalar_mul(ge[:, 0:4], ct[:, 0:4], acol)
                else:
                    nc.gpsimd.tensor_scalar_mul(ge[:, 0:B], ct[:, 0:B], acol)
                    nc.gpsimd.tensor_scalar_mul(ge[:, B:2 * B],
                                                ct[:, 512:512 + B], acol)
                    nc.gpsimd.tensor_scalar_mul(ge[:, 2 * B:3 * B],
                                                ct[:, 2176 - B:2176], acol)
                    nc.gpsimd.tensor_scalar_mul(ge[:, 3 * B:4 * B],
                                                ct[:, 2304 - B:2304], acol)
                stash[ch] = (xg, ct, ge)

        def down_chunk(ch, ci):
            xt, ct, ge = stash[ch]
            g, chi = divmod(ch, GRP)
            if chi == 0 and ci == 0:
                ot = op.tile([NB, GRP * COLS], F16, tag="ot")
                stash[("ot", g)] = ot
            ot = stash[("ot", g)]
            ob = chi * COLS
            c0, w = CHUNKS[ci]
            pso = po.tile([XW, 512], F32, tag="po")
            ranges = [(max(s0, c0), min(s1, c0 + w), v) for (s0, s1, v) in SPLITS]
            ranges = [r for r in ranges if r[0] < r[1]]
            ranges.sort(key=lambda r: r[2] != 1)   # interior first
            xb = chi * COLS
            if "down_mm" in ablate:
                nc.tensor.matmul(pso[:, 0:16], wt[:, W_H[1]:W_H[1] + NBP],
                                 xt[:, xb + c0:xb + c0 + 16],
                                 start=True, stop=True)
                ranges = []
            for (a0, a1, v) in ranges:
                l0, l1 = a0 - c0, a1 - c0
                nc.tensor.matmul(pso[:, l0:l1],
                                 wt[:, W_H[v]:W_H[v] + NBP],
                                 xt[:, xb + a0:xb + a1],
                                 start=True, stop=False)
                if v == 1:
                    nc.tensor.matmul(pso[:, l0:l1],
                                     uch_d[0:UW, ch, 0:NBP],
                                     ct[:, 2 * c0 + l0:2 * c0 + l1],
                                     start=False, stop=False)
                    nc.tensor.matmul(pso[:, l0:l1],
                                     uch_d[0:UW, ch, NBP:2 * NBP],
                                     ct[:, 2 * c0 + w + l0:2 * c0 + w + l1],
                                     start=False, stop=True)
                else:
                    goff = 0 if v == 0 else 2 * B
                    nc.tensor.matmul(pso[:, l0:l1],
                                     wt[0:UW, W_DE[v]:W_DE[v] + NBP],
                                     ge[:, goff:goff + B],
                                     start=False, stop=False)
                    nc.tensor.matmul(pso[:, l0:l1],
                                     wt[0:UW, W_DO[v]:W_DO[v] + NBP],
                                     ge[:, goff + B:goff + 2 * B],
                                     start=False, stop=True)
            if "dve" not in ablate:
                nc.vector.tensor_scalar(
                    ot[:, ob + c0:ob + c0 + w], pso[0:NB, 0:w],
                    tb[0:NB, 2 * CPC + ch:2 * CPC + ch + 1], None,
                    op0=mybir.AluOpType.add)
            else:
                nc.vector.tensor_scalar(
                    ot[:, ob:ob + 4], pso[0:NB, 0:4],
                    tb[0:NB, 2 * CPC + ch:2 * CPC + ch + 1], None,
                    op0=mybir.AluOpType.add)
            if ci == len(CHUNKS) - 1:
                del stash[ch]
                if chi == GRP - 1:
                    if "out_dma" not in ablate:
                        nc.sync.dma_start(o_d[g], ot[:])
                    del stash[("ot", g)], stash[("xg", g)]

        def whole_pipeline():
            for ch in range(6):
                build_uch(ch)
            for ch in range(CPC + 1):
                for ci in range(len(CHUNKS)):
                    if ch < CPC:
                        if ci == 0 and ch + 6 < CPC:
                            build_uch(ch + 6)
                        up_chunk(ch, ci)
                    if ch >= 1:
                        down_chunk(ch - 1, ci)

        if repeat:
            # timing-loop build: run the whole pipeline `repeat` times
            with tc.For_i(0, repeat, 1):
                whole_pipeline()
        else:
            whole_pipeline()
    nc.compile()
    return nc


_NC_CACHE = None


def _get_nc():
    global _NC_CACHE
    if _NC_CACHE is None:
        _NC_CACHE = build_nc()
    return _NC_CACHE


def make_in_maps(x, up_filter, down_filter, alpha, beta):
    w_pack, u_base, S = _host_constants(up_filter, down_filter)
    a = np.exp(alpha.astype(np.float64))
    rb = 1.0 / (np.exp(beta.astype(np.float64)) + 1e-9)
    in_maps = []
    for core in range(NCORES):
        ch0 = core * CPC
        x_dev = _prep_x_core(x[:, ch0:ch0 + CPC, :])
        tbl = np.zeros((XW, TBLCOLS), np.float32)
        tbl[:, 0:CPC] = a[ch0:ch0 + CPC] / np.pi
        tbl[127, 0:CPC] = 1.0
        tbl[:, CPC:2 * CPC] = -rb[ch0:ch0 + CPC] / 2.0
        tbl[:, 2 * CPC:3 * CPC] = (rb[ch0:ch0 + CPC] / 2.0) * S
        tbl[:, 3 * CPC] = -np.pi
        in_maps.append({"x_dev": x_dev, "w_pack": w_pack, "u_base": u_base,
                        "tbl": np.ascontiguousarray(tbl)})
    return in_maps


def unshard(results):
    out = np.empty((B, C, T), np.float32)
    for core in range(NCORES):
        ch0 = core * CPC
        od = np.asarray(results[core]["out_dev"]).astype(np.float32)
        od = od.reshape(NGRP, NB, GRP, COLS).transpose(0, 2, 1, 3)
        blk = od.reshape(CPC, NB, NBLK, B).transpose(3, 0, 2, 1)
        out[:, ch0:ch0 + CPC, :] = blk.reshape(B, CPC, NBLK * NB)[:, :, :T]
    return out


def kernel(x, up_filter, down_filter, alpha, beta):
    x = np.asarray(x, np.float32)
    up_filter = np.asarray(up_filter, np.float32)
    down_filter = np.asarray(down_filter, np.float32)
    alpha = np.asarray(alpha, np.float32)
    beta = np.asarray(beta, np.float32)

    in_maps = make_in_maps(x, up_filter, down_filter, alpha, beta)
    nc = _get_nc()
    res = run_bass_kernel_spmd(nc, in_maps, core_ids=list(range(NCORES)))
    return unshard(res.results)


if __name__ == "__main__":
    rng = np.random.default_rng(0)
    x = rng.standard_normal((B, C, T), dtype=np.float32)
    uf = rng.standard_normal(K).astype(np.float32)
    df = rng.standard_normal(K).astype(np.float32)
    al = (0.1 * rng.standard_normal(C)).astype(np.float32)
    be = (0.1 * rng.standard_normal(C)).astype(np.float32)
    o = kernel(x, uf, df, al, be)
    print("kernel ran, out shape", o.shape)



# revision 36
# speedup vs baseline: 1.2499x; 1.2499x over previous
"""Fused BigVGAN Activation1d (upsample2x -> SnakeBeta -> downsample2x) on
8 Trainium2 NeuronCores.

Strategy (validated in golden.py):
  - Channel-sharded SPMD: 512 channels / 8 cores = 64 channels per core,
    16 batch rows each, T=8192. No cross-core communication.
  - All FIRs are banded-matrix matmuls on the TensorEngine in a
    time-in-partitions layout; the host bakes overlapped clamped x
    windows (stride NB=115, length 128, row 127 = constant 1.0) so the
    device never transposes and never special-cases edges.
  - Math:  act = up + rb*sin^2(a*up),  a=exp(alpha), rb=1/(exp(beta)+1e-9)
        out = D(act) = H(x) + D((rb/2)*(-cos(2a*up))) + (rb/2)*sum(d)
    The up-matmul emits w' = (a/pi)*u + 96.25 (per-channel stationaries,
    offset via the const row).  W' lives in [64,128) so frac(w') is the
    low 17 mantissa bits: one DVE bitwise_and extracts it; ScalarE
    computes c = Sin(2pi*frac - pi) = -cos(2a*u); GPSIMD applies the
    per-channel rb/2; H/D matmuls accumulate in PSUM; the final
    PSUM->SBUF copy adds (rb/2)*sum(d).
"""
import numpy as np
from contextlib import ExitStack

import concourse.bacc as bacc
import concourse.tile as tile
from concourse import mybir
from concourse.bass_utils import run_bass_kernel_spmd

# ---- problem geometry (hardcoded per spec) --------------------------------
B, C, T = 16, 512, 8192
NCORES = 8
CPC = C // NCORES          # 64 channels per core
K = 12
NB = 115                   # out samples per block
NBLK = (T + NB - 1) // NB  # 72
XW, UW = 128, 121
XOFF, UOFF = -6, -3
COLS = NBLK * B            # 1152 free columns per channel (block-major, batch)
OFFSET = 192.25            # binade [128,256): frac = the low int16 of the f32
FRACBITS = 16

F16 = mybir.dt.float16
F32 = mybir.dt.float32
I32 = mybir.dt.int32
I16 = mybir.dt.int16

# column chunking for PSUM tiles
CHUNKS = [(0, 512), (512, 512), (1024, 128)]
# H/D stationary variant splits over columns: (start, end, variant)
SPLITS = [(0, B, 0), (B, (NBLK - 1) * B, 1), ((NBLK - 1) * B, COLS, 2)]
# shared w_pack column offsets: H[3] | De[3] | Do[3], each padded to 128
# columns so LDWEIGHTS runs with a full 128-wide stationary (FWL-eligible)
NBP = 128
W_H = [NBP * k for k in range(3)]
W_DE = [3 * NBP + NBP * k for k in range(3)]
W_DO = [6 * NBP + NBP * k for k in range(3)]
WCOLS = 9 * NBP
# base (unscaled) up stationaries, shared: ue | uo, padded to 128 cols
UWP = 128
UBCOLS = 2 * UWP
# tbl columns: a/pi | A=rb/2 | BS | -pi
TBLCOLS = 3 * CPC + 1


# ---------------------------------------------------------------------------
# host-side constant builders (validated in golden.py)
# ---------------------------------------------------------------------------

def _phase_filters(up_filter):
    f = up_filter.astype(np.float64)
    fe = np.array([2.0 * f[11 - 2 * j] for j in range(6)])
    fo = np.array([2.0 * f[10 - 2 * j] for j in range(6)])
    return fe, fo


def _build_up_stationaries(fe, fo):
    W_ue = np.zeros((XW, UW))
    W_uo = np.zeros((XW, UW))
    for q in range(UW):
        for k in range(6):
            W_ue[q + k, q] += fe[k]
            W_uo[q + k + 1, q] += fo[k]
    return W_ue, W_uo


def _build_down_stationaries(d_full, block):
    D_We = np.zeros((XW, NB))
    D_Wo = np.zeros((XW, NB))
    u0 = NB * block + UOFF
    for o in range(NB):
        n = NB * block + o
        if n >= T:
            continue
        for t in range(K):
            c = min(max(2 * n + t - 5, 0), 2 * T - 1)
            q = c // 2 - u0
            if c % 2 == 0:
                D_We[q, o] += d_full[t]
            else:
                D_Wo[q, o] += d_full[t]
    return D_We, D_Wo


def _build_h_stationary(fe, fo, d_full, block):
    H = np.zeros((XW, NB))
    x0 = NB * block + XOFF
    for o in range(NB):
        n = NB * block + o
        if n >= T:
            continue
        for t in range(K):
            c = min(max(2 * n + t - 5, 0), 2 * T - 1)
            u = c // 2
            if c % 2 == 0:
                base, taps = u - 3, fe
            else:
                base, taps = u - 2, fo
            for k in range(6):
                xg = min(max(base + k, -5), T + 4)
                H[xg - x0, o] += d_full[t] * taps[k]
    return H


def _host_constants(up_filter, down_filter):
    fe, fo = _phase_filters(up_filter)
    d_full = down_filter.astype(np.float64)
    w = np.zeros((XW, WCOLS))
    for k, blk in enumerate((0, 1, NBLK - 1)):
        w[:, W_H[k]:W_H[k] + NB] = _build_h_stationary(fe, fo, d_full, blk)
        De, Do = _build_down_stationaries(d_full, blk)
        w[:, W_DE[k]:W_DE[k] + NB] = De
        w[:, W_DO[k]:W_DO[k] + NB] = Do
    W_ue, W_uo = _build_up_stationaries(fe, fo)
    ub = np.zeros((XW, UBCOLS))
    ub[:, :UW] = W_ue
    ub[:, UWP:UWP + UW] = W_uo
    ub[127, :] = OFFSET    # scaled by tbl row 127 == 1.0 on the uch build
    S = float(np.sum(d_full))
    return (np.ascontiguousarray(w.astype(np.float16)),
            np.ascontiguousarray(ub.astype(np.float16)), S)


GRP = 1                     # channels per DMA group
NGRP = CPC // GRP           # groups per core


def _prep_x_core(x_core):
    """x_core: [B, CPC, T] f32 -> x_dev [NGRP, XW, GRP, COLS] fp16 windows,
    grouped so one DMA loads GRP channels."""
    idx = np.arange(NBLK)[:, None] * NB + XOFF + np.arange(XW)[None, :]
    idx = np.clip(idx, 0, T - 1)
    wins = x_core[:, :, idx]                      # [B, CPC, NBLK, XW]
    x_dev = wins.transpose(1, 3, 2, 0).reshape(CPC, XW, COLS).astype(np.float16)
    x_dev[:, 127, :] = np.float16(1.0)
    x_dev = x_dev.reshape(NGRP, GRP, XW, COLS).transpose(0, 2, 1, 3)
    return np.ascontiguousarray(x_dev)


# ---------------------------------------------------------------------------
# device kernel
# ---------------------------------------------------------------------------

def build_nc(repeat=0, ablate=()):
    """ablate: subset of {"in_dma","out_dma","act","up_mm","down_mm","dve"}
    — drops that component (garbage numerics) for HW bottleneck attribution."""
    ablate = frozenset(ablate)
    nc = bacc.Bacc("TRN2", target_bir_lowering=False, debug=False,
                   num_devices=NCORES)
    x_d = nc.declare_dram_parameter("x_dev", [NGRP, XW, GRP * COLS], F16,
                                    isOutput=False)
    w_d = nc.declare_dram_parameter("w_pack", [XW, WCOLS], F16, isOutput=False)
    u_d = nc.declare_dram_parameter("u_base", [XW, UBCOLS], F16, isOutput=False)
    t_d = nc.declare_dram_parameter("tbl", [XW, TBLCOLS], F32, isOutput=False)
    o_d = nc.declare_dram_parameter("out_dev", [NGRP, NB, GRP * COLS], F16,
                                    isOutput=True)

    SIN_SCALE = float(2.0 * np.pi / (1 << FRACBITS))

    with tile.TileContext(nc) as tc, ExitStack() as ctx:
        wp = ctx.enter_context(tc.tile_pool(name="wp", bufs=1))
        xp = ctx.enter_context(tc.tile_pool(name="xp", bufs=8))
        cp = ctx.enter_context(tc.tile_pool(name="cp", bufs=4))
        gp = ctx.enter_context(tc.tile_pool(name="gp", bufs=4))
        op = ctx.enter_context(tc.tile_pool(name="op", bufs=4))
        pu = ctx.enter_context(tc.tile_pool(name="pu", bufs=3, space="PSUM"))
        po = ctx.enter_context(tc.tile_pool(name="po", bufs=2, space="PSUM"))

        wt = wp.tile([XW, WCOLS], F16)
        nc.sync.dma_start(wt[:], w_d[:])
        ub = wp.tile([XW, UBCOLS], F16)
        nc.sync.dma_start(ub[:], u_d[:])
        tb = wp.tile([XW, TBLCOLS], F32)
        nc.sync.dma_start(tb[:], t_d[:])

        # per-channel scaled up stationaries: rows 0..126 = (a/pi)*base,
        # row 127 = OFFSET (tbl row 127 of the a/pi block is 1.0)
        uch = wp.tile([XW, CPC, UBCOLS], F16)
        uch_d = wp.tile([XW, CPC, 2 * NBP], F16)

        def build_uch(ch):
            # just-in-time per-channel stationary scaling, all on the
            # otherwise-idle GPSIMD so ACT/DVE queues stay clean; issued
            # several channels ahead of first use.
            n = 4 if "build" in ablate else None
            nc.gpsimd.tensor_scalar_mul(uch[:, ch, 0:n or UBCOLS],
                                        ub[:, 0:n or UBCOLS],
                                        tb[:, ch:ch + 1])
            nc.vector.tensor_scalar_mul(
                uch_d[:, ch, 0:n or NBP],
                wt[:, W_DE[1]:W_DE[1] + (n or NBP)],
                tb[:, CPC + ch:CPC + ch + 1])
            nc.vector.tensor_scalar_mul(
                uch_d[:, ch, NBP:NBP + (n or NBP)],
                wt[:, W_DO[1]:W_DO[1] + (n or NBP)],
                tb[:, CPC + ch:CPC + ch + 1])

        # Chunk-granular software pipeline: PE alternates up-chunk(ch,k)
        # with down-chunk(ch-1,k), so ScalarE sin of chunk k overlaps the
        # PE's down-stage matmuls and no engine waits a full channel.
        stash = {}

        def up_chunk(ch, ci):
            c0, w = CHUNKS[ci]
            g, chi = divmod(ch, GRP)
            if chi == 0 and ci == 0:
                xg = xp.tile([XW, GRP * COLS], F16, tag="xt")
                if "in_dma" not in ablate:
                    nc.sync.dma_start(xg[:], x_d[g])
                else:
                    nc.gpsimd.memset(xg[:, 0:4], 1.0)
                stash[("xg", g)] = xg
            if ci == 0:
                ct = cp.tile([UW, 2 * COLS], F16, tag="ct")
                stash[ch] = (stash[("xg", g)], ct, None)
            xg, ct, _ = stash[ch]
            xb = chi * COLS
            ps = pu.tile([XW, 1024], F32, tag="pu")
            if "up_mm" not in ablate:
                nc.tensor.matmul(ps[:, 0:w], uch[:, ch, 0:UWP],
                                 xg[:, xb + c0:xb + c0 + w],
                                 start=True, stop=True)
                nc.tensor.matmul(ps[:, w:2 * w], uch[:, ch, UWP:UBCOLS],
                                 xg[:, xb + c0:xb + c0 + w],
                                 start=True, stop=True)
            else:
                nc.tensor.matmul(ps[:, 0:16], uch[:, ch, 0:UWP],
                                 xg[:, xb + c0:xb + c0 + 16],
                                 start=True, stop=True)
            lo16 = ps[0:UW, 0:2 * w].bitcast(I16)[:, 0::2]
            if "act" not in ablate:
                nc.scalar.activation(ct[:, 2 * c0:2 * c0 + 2 * w], lo16,
                                     mybir.ActivationFunctionType.Sin,
                                     bias=0.0, scale=SIN_SCALE)
            else:
                nc.scalar.activation(ct[:, 2 * c0:2 * c0 + 8], lo16[:, 0:8],
                                     mybir.ActivationFunctionType.Sin,
                                     bias=0.0, scale=SIN_SCALE)
            if ci == len(CHUNKS) - 1:
                # edge-block g = -(rb/2)*c: [ge_j0|go_j0|ge_j71|go_j71]
                ge = gp.tile([UW, 4 * B], F16, tag="ge")
                acol = tb[0:UW, CPC + ch:CPC + ch + 1]
                if "ge" in ablate:
                    nc.gpsimd.tensor_scalar_mul(ge[:, 0:4], ct[:, 0:4], acol)
                else:
                    nc.gpsimd.tensor_scalar_mul(ge[:, 0:B], ct[:, 0:B], acol)
                    nc.gpsimd.tensor_scalar_mul(ge[:, B:2 * B],
                                                ct[:, 512:512 + B], acol)
                    nc.gpsimd.tensor_scalar_mul(ge[:, 2 * B:3 * B],
                                                ct[:, 2176 - B:2176], acol)
                    nc.gpsimd.tensor_scalar_mul(ge[:, 3 * B:4 * B],
                                                ct[:, 2304 - B:2304], acol)
                stash[ch] = (xg, ct, ge)

        def down_chunk(ch, ci):
            xt, ct, ge = stash[ch]
            g, chi = divmod(ch, GRP)
            if chi == 0 and ci == 0:
                ot = op.tile([NB, GRP * COLS], F16, tag="ot")
                stash[("ot", g)] = ot
            ot = stash[("ot", g)]
            ob = chi * COLS
            c0, w = CHUNKS[ci]
            pso = po.tile([XW, 512], F32, tag="po")
            ranges = [(max(s0, c0), min(s1, c0 + w), v) for (s0, s1, v) in SPLITS]
            ranges = [r for r in ranges if r[0] < r[1]]
            ranges.sort(key=lambda r: r[2] != 1)   # interior first
            xb = chi * COLS
            if "down_mm" in ablate:
                nc.tensor.matmul(pso[:, 0:16], wt[:, W_H[1]:W_H[1] + NBP],
                                 xt[:, xb + c0:xb + c0 + 16],
                                 start=True, stop=True)
                ranges = []
            for (a0, a1, v) in ranges:
                l0, l1 = a0 - c0, a1 - c0
                nc.tensor.matmul(pso[:, l0:l1],
                                 wt[:, W_H[v]:W_H[v] + NBP],
                                 xt[:, xb + a0:xb + a1],
                                 start=True, stop=False)
                if v == 1:
                    nc.tensor.matmul(pso[:, l0:l1],
                                     uch_d[0:UW, ch, 0:NBP],
                                     ct[:, 2 * c0 + l0:2 * c0 + l1],
                                     start=False, stop=False)
                    nc.tensor.matmul(pso[:, l0:l1],
                                     uch_d[0:UW, ch, NBP:2 * NBP],
                                     ct[:, 2 * c0 + w + l0:2 * c0 + w + l1],
                                     start=False, stop=True)
                else:
                    goff = 0 if v == 0 else 2 * B
                    nc.tensor.matmul(pso[:, l0:l1],
                                     wt[0:UW, W_DE[v]:W_DE[v] + NBP],
                                     ge[:, goff:goff + B],
                                     start=False, stop=False)
                    nc.tensor.matmul(pso[:, l0:l1],
                                     wt[0:UW, W_DO[v]:W_DO[v] + NBP],
                                     ge[:, goff + B:goff + 2 * B],
                                     start=False, stop=True)
            if "dve" not in ablate:
                nc.vector.tensor_scalar(
                    ot[:, ob + c0:ob + c0 + w], pso[0:NB, 0:w],
                    tb[0:NB, 2 * CPC + ch:2 * CPC + ch + 1], None,
                    op0=mybir.AluOpType.add)
            else:
                nc.vector.tensor_scalar(
                    ot[:, ob:ob + 4], pso[0:NB, 0:4],
                    tb[0:NB, 2 * CPC + ch:2 * CPC + ch + 1], None,
                    op0=mybir.AluOpType.add)
            if ci == len(CHUNKS) - 1:
                del stash[ch]
                if chi == GRP - 1:
                    if "out_dma" not in ablate:
                        nc.gpsimd.dma_start(o_d[g], ot[:])
                    del stash[("ot", g)], stash[("xg", g)]

        def whole_pipeline():
            for ch in range(6):
                build_uch(ch)
            for ch in range(CPC + 1):
                for ci in range(len(CHUNKS)):
                    if ch < CPC:
                        if ci == 0 and ch + 6 < CPC:
                            build_uch(ch + 6)
                        up_chunk(ch, ci)
                    if ch >= 1:
                        down_chunk(ch - 1, ci)

        if repeat:
            # timing-loop build: run the whole pipeline `repeat` times
            with tc.For_i(0, repeat, 1):
                whole_pipeline()
        else:
            whole_pipeline()
    nc.compile()
    return nc


_NC_CACHE = None


def _get_nc():
    global _NC_CACHE
    if _NC_CACHE is None:
        _NC_CACHE = build_nc()
    return _NC_CACHE


def make_in_maps(x, up_filter, down_filter, alpha, beta):
    w_pack, u_base, S = _host_constants(up_filter, down_filter)
    a = np.exp(alpha.astype(np.float64))
    rb = 1.0 / (np.exp(beta.astype(np.float64)) + 1e-9)
    in_maps = []
    for core in range(NCORES):
        ch0 = core * CPC
        x_dev = _prep_x_core(x[:, ch0:ch0 + CPC, :])
        tbl = np.zeros((XW, TBLCOLS), np.float32)
        tbl[:, 0:CPC] = a[ch0:ch0 + CPC] / np.pi
        tbl[127, 0:CPC] = 1.0
        tbl[:, CPC:2 * CPC] = -rb[ch0:ch0 + CPC] / 2.0
        tbl[:, 2 * CPC:3 * CPC] = (rb[ch0:ch0 + CPC] / 2.0) * S
        tbl[:, 3 * CPC] = -np.pi
        in_maps.append({"x_dev": x_dev, "w_pack": w_pack, "u_base": u_base,
                        "tbl": np.ascontiguousarray(tbl)})
    return in_maps


def unshard(results):
    out = np.empty((B, C, T), np.float32)
    for core in range(NCORES):
        ch0 = core * CPC
        od = np.asarray(results[core]["out_dev"]).astype(np.float32)
        od = od.reshape(NGRP, NB, GRP, COLS).transpose(0, 2, 1, 3)
        blk = od.reshape(CPC, NB, NBLK, B).transpose(3, 0, 2, 1)
        out[:, ch0:ch0 + CPC, :] = blk.reshape(B, CPC, NBLK * NB)[:, :, :T]
    return out


def kernel(x, up_filter, down_filter, alpha, beta):
    x = np.asarray(x, np.float32)
    up_filter = np.asarray(up_filter, np.float32)
    down_filter = np.asarray(down_filter, np.float32)
    alpha = np.asarray(alpha, np.float32)
    beta = np.asarray(beta, np.float32)

    in_maps = make_in_maps(x, up_filter, down_filter, alpha, beta)
    nc = _get_nc()
    res = run_bass_kernel_spmd(nc, in_maps, core_ids=list(range(NCORES)))
    return unshard(res.results)


if __name__ == "__main__":
    rng = np.random.default_rng(0)
    x = rng.standard_normal((B, C, T), dtype=np.float32)
    uf = rng.standard_normal(K).astype(np.float32)
    df = rng.standard_normal(K).astype(np.float32)
    al = (0.1 * rng.standard_normal(C)).astype(np.float32)
    be = (0.1 * rng.standard_normal(C)).astype(np.float32)
    o = kernel(x, uf, df, al, be)
    print("kernel ran, out shape", o.shape)

